# revision 46
# baseline (speedup 1.0000x reference)
"""Trainium2 Bass kernel for nn_MultiHeadAttention (B=32, S=1024, E=64, H=8, D=64).

Strategy (per core; batch-parallel over 8 cores, 4 batches each):
  - Host-side numpy prep: permute w_qkv columns into per-head Q/K/V blocks,
    transpose x to xT (head-dim on partitions), duplicate operands across
    both 64-partition halves so pairs of K=64 matmuls run concurrently via
    PE row tiling. V is pre-scaled by 1/8 (the post-softmax scale).
  - On chip, everything stays in "transposed" layouts so no PE transposes
    are needed anywhere:
      qT/kT:   [2 heads * 64 d, 1024 nq]  (4 tiles per batch)
      V:       [128 nk-chunk, 8 heads * (64 v | 1)]  ones col => rowsums
      E^T:     [128 nk, 2 heads * 512 nq] per chunk -> exp (ScalarE) -> P^T
      P^T @ [V|1]: accumulates [65, 512] per head: rows 0..63 = (P V)/8,
                   row 64 = rowsum.  softmax normalization = multiply by
                   broadcast reciprocal of row 64 (no max subtraction:
                   energies are |E| < ~60, exp stays in fp32 range; softmax
                   is shift invariant so this matches the reference).
      proj:    per-head K=64 matmuls accumulate yT [64 e, nq]; output is
               returned transposed and fixed up on host.
  - Reciprocals are batched: rowsum rows are gathered via SBUF->SBUF DMA
    into a [16, 512] tile (one DVE reciprocal per batch), results DMA
    broadcast back across 64 partitions.
"""

import os
import sys

import numpy as np

_TRN_REPO = "/opt/trn_rl_repo"
if _TRN_REPO not in sys.path:
    sys.path.insert(0, _TRN_REPO)

B, S, E, H, D = 32, 1024, 64, 8, 64
HID = H * D  # 512
N_CORES = 8
NQH = 512  # nq half processed per psum tile


def build_nc(bb=B // N_CORES, dt_e="f32r", dt_pv="f32r", dt_qkv="f32r", dt_proj="f32r",
             reps=1):
    """Build the per-core Bass kernel. bb = batches per core."""
    import concourse.bass as bass
    import concourse.mybir as mybir
    import concourse.tile as tile
    from concourse import bacc
    from contextlib import ExitStack

    f32 = mybir.dt.float32
    f32r = mybir.dt.float32r
    bf16 = mybir.dt.bfloat16
    Exp = mybir.ActivationFunctionType.Exp

    def dt_of(key):
        return f32r if key == "f32r" else f32

    dte, dtpv, dtqkv, dtproj = dt_of(dt_e), dt_of(dt_pv), dt_of(dt_qkv), dt_of(dt_proj)

    nc = bacc.Bacc(None, target_bir_lowering=False)

    # ---- DRAM I/O (host-prepped layouts) ----
    xT_d = nc.dram_tensor("xT", [bb, 128, S], dtqkv, kind="ExternalInput")
    wq_d = nc.dram_tensor("wq", [128, HID], dtqkv, kind="ExternalInput")
    wk_d = nc.dram_tensor("wk", [128, HID], dtqkv, kind="ExternalInput")
    wv_d = nc.dram_tensor("wv", [128, HID], dtqkv, kind="ExternalInput")  # pre /8
    bqk_d = nc.dram_tensor("bqk", [128, 8], f32, kind="ExternalInput")
    bv_d = nc.dram_tensor("bv", [HID], f32, kind="ExternalInput")  # pre /8
    wp_d = nc.dram_tensor("wp", [64, H, E], dtproj, kind="ExternalInput")
    bp_d = nc.dram_tensor("bp", [E, 1], f32, kind="ExternalInput")
    yT_d = nc.dram_tensor("yT", [bb, E, S], f32, kind="ExternalOutput")

    with tile.TileContext(nc) as tc, ExitStack() as ctx:
        wpool = ctx.enter_context(tc.tile_pool(name="weights", bufs=1))
        qkpool = ctx.enter_context(tc.tile_pool(name="qk", bufs=2))
        vpool = ctx.enter_context(tc.tile_pool(name="v", bufs=2))
        ptpool = ctx.enter_context(tc.tile_pool(name="pt", bufs=4))
        ovpool = ctx.enter_context(tc.tile_pool(name="ov", bufs=16))
        rbpool = ctx.enter_context(tc.tile_pool(name="rb", bufs=3))
        miscpool = ctx.enter_context(tc.tile_pool(name="misc", bufs=2))
        psum_e = ctx.enter_context(tc.tile_pool(name="psum_e", bufs=2, space="PSUM"))
        psum_s = ctx.enter_context(tc.tile_pool(name="psum_s", bufs=4, space="PSUM"))
        drampool = ctx.enter_context(tc.tile_pool(name="dram", bufs=2, space="DRAM"))

        def alloc_batch(bi, b):
            xT_sb = qkpool.tile([128, S], dtqkv, tag="xT", name=f"xT_{b}")
            nc.sync.dma_start(out=xT_sb[:, 0:NQH], in_=xT_d[bi][:, 0:NQH])
            nc.sync.dma_start(out=xT_sb[:, NQH:S], in_=xT_d[bi][:, NQH:S])
            qT = [qkpool.tile([128, S], dte, tag=f"qT{t}", name=f"qT{t}_{b}") for t in range(4)]
            kT = [qkpool.tile([128, S], dte, tag=f"kT{t}", name=f"kT{t}_{b}") for t in range(4)]
            v_nat = [vpool.tile([128, H * 65], dtpv, tag=f"v{c}", name=f"v{c}_{b}") for c in range(8)]
            rsb = []
            for hf in range(2):
                t = miscpool.tile([97, 2 * NQH], f32, tag=f"rsb{hf}", name=f"rsb_{b}_{hf}")
                nc.vector.memset(t, 1.0)
                rsb.append(t)
            return dict(bi=bi, b=b, xT=xT_sb, qT=qT, kT=kT, v=v_nat, ov={}, rsb=rsb)

        def emit_qk_pair(st, qki, tp):
            w_sb = (wq_sb, wk_sb)[qki]
            dst = (st["qT"], st["kT"])[qki]
            xT_sb, b = st["xT"], st["b"]
            for half in range(2):
                nq = slice(half * NQH, (half + 1) * NQH)
                ps_e = psum_s.tile([128, NQH], f32, tag="small", name=f"psqkv_e{b}_{qki}{tp}{half}")
                ps_o = psum_s.tile([128, NQH], f32, tag="small", name=f"psqkv_o{b}_{qki}{tp}{half}")
                nc.tensor.matmul(ps_e, w_sb[0:64, 128 * tp : 128 * (tp + 1)], xT_sb[0:64, nq])
                nc.tensor.matmul(ps_o, w_sb[64:128, 128 * (tp + 1) : 128 * (tp + 2)], xT_sb[64:128, nq])
                nc.vector.tensor_scalar_add(
                    dst[tp][:, nq], ps_e, bqk_sb[:, qki * 4 + tp : qki * 4 + tp + 1]
                )
                nc.vector.tensor_scalar_add(
                    dst[tp + 1][:, nq], ps_o, bqk_sb[:, qki * 4 + tp + 1 : qki * 4 + tp + 2]
                )

        def emit_v_pair(st, cp):
            xT_sb, v_nat, b = st["xT"], st["v"], st["b"]
            ps_e = psum_s.tile([128, HID], f32, tag="small", name=f"psv_e{b}_{cp}")
            ps_o = psum_s.tile([128, HID], f32, tag="small", name=f"psv_o{b}_{cp}")
            nc.tensor.matmul(ps_e, xT_sb[0:64, 128 * cp : 128 * (cp + 1)], wv_sb[0:64, :])
            nc.tensor.matmul(ps_o, xT_sb[64:128, 128 * (cp + 1) : 128 * (cp + 2)], wv_sb[64:128, :])
            for c, pss in ((cp, ps_e), (cp + 1, ps_o)):
                vdst = v_nat[c].rearrange("p (h c65) -> p h c65", c65=65)
                nc.vector.tensor_tensor(
                    vdst[:, :, 0:64],
                    pss.rearrange("p (h d) -> p h d", d=64),
                    bv_sb.rearrange("p (h d) -> p h d", d=64),
                    mybir.AluOpType.add,
                )
                nc.vector.tensor_copy(vdst[:, :, 64], ones_sb)

        def emit_qkv_group(st, g):
            # startup-friendly order: heads 0-3 weights, all V, heads 4-7
            if g == 0:
                emit_qk_pair(st, 0, 0)
                emit_qk_pair(st, 1, 0)
            elif g == 1:
                emit_v_pair(st, 0)
                emit_v_pair(st, 2)
            elif g == 2:
                emit_v_pair(st, 4)
                emit_v_pair(st, 6)
            else:
                emit_qk_pair(st, 0, 2)
                emit_qk_pair(st, 1, 2)

        def emit_attention_unit(st, hp, half):
            qT, kT, v_nat, b = st["qT"], st["kT"], st["v"], st["b"]
            nq = slice(half * NQH, (half + 1) * NQH)
            oT_e = psum_s.tile([65, NQH], f32, tag="small", name=f"oTe_{b}_{hp}_{half}")
            oT_o = psum_s.tile([65, NQH], f32, tag="small", name=f"oTo_{b}_{hp}_{half}")
            for c in range(8):
                eT = psum_e.tile([128, 2 * NQH], f32, tag="eT", name=f"eT_{b}_{hp}_{half}_{c}")
                nc.tensor.matmul(
                    eT[:, 0:NQH], kT[hp][0:64, 128 * c : 128 * (c + 1)], qT[hp][0:64, nq]
                )
                nc.tensor.matmul(
                    eT[:, NQH : 2 * NQH],
                    kT[hp][64:128, 128 * c : 128 * (c + 1)],
                    qT[hp][64:128, nq],
                )
                pt = ptpool.tile([128, 2 * NQH], dtpv, tag="pt", name=f"pt_{b}_{hp}_{half}_{c}")
                nc.scalar.activation(pt, eT, Exp)
                nc.tensor.matmul(
                    oT_e,
                    v_nat[c][:, (2 * hp) * 65 : (2 * hp) * 65 + 65],
                    pt[:, 0:NQH],
                    start=(c == 0),
                    stop=(c == 7),
                )
                nc.tensor.matmul(
                    oT_o,
                    v_nat[c][:, (2 * hp + 1) * 65 : (2 * hp + 1) * 65 + 65],
                    pt[:, NQH : 2 * NQH],
                    start=(c == 0),
                    stop=(c == 7),
                )
            rsb = st["rsb"][half]
            for par, oT in ((0, oT_e), (1, oT_o)):
                h = 2 * hp + par
                t = ovpool.tile([65, NQH], dtproj, tag="ov", name=f"ov_{b}_{h}_{half}")
                nc.vector.tensor_copy(t, oT)
                st["ov"][(h, half)] = t
                # gather rowsum rows onto partitions {0,32,64,96} (the only
                # legal SBUF AP start partitions), 2 col-blocks each
                p, blk = 32 * (h // 2), h % 2
                nc.sync.dma_start(
                    out=rsb[p : p + 1, blk * NQH : (blk + 1) * NQH],
                    in_=t[64:65, :].bitcast(f32),
                )

        def emit_norm_proj(st, half, yT_sb, tail=False):
            ov, b, bi = st["ov"], st["b"], st["bi"]
            rsb = st["rsb"][half]
            rcpb = miscpool.tile([97, 2 * NQH], f32, tag="rcpb", name=f"rcpb_{b}_{half}")
            rscr = miscpool.tile([97, 2 * NQH], f32, tag="rscr", name=f"rscr_{b}_{half}", bufs=1)
            # full contiguous partition range: only rows {0,32,64,96} hold real
            # rowsums, the rest compute garbage that is never read
            nc.vector.reciprocal_approx_accurate(rcpb, rsb, rscr)
            rcpd = drampool.tile([97, 2 * NQH], f32, tag=f"rcpd{half}", name=f"rcpd_{b}_{half}")
            dma_eng = nc.scalar if tail else nc.sync
            dma_eng.dma_start(out=rcpd, in_=rcpb)
            nq = slice(half * NQH, (half + 1) * NQH)
            for hp in range(4):
                recB = rbpool.tile([64, 2 * NQH], f32, tag="recB", name=f"recB_{b}_{hp}_{half}")
                dma_eng.dma_start(
                    out=recB, in_=rcpd[32 * hp : 32 * hp + 1, :].partition_broadcast(64)
                )
                for par in range(2):
                    h = 2 * hp + par
                    nc.vector.tensor_tensor(
                        ov[(h, half)][0:64, :],
                        ov[(h, half)][0:64, :],
                        recB[:, par * NQH : (par + 1) * NQH],
                        mybir.AluOpType.mult,
                    )
            yT_ps = psum_s.tile([E, NQH], f32, tag="small", name=f"yTps_{b}_{half}")
            for h in range(H):
                nc.tensor.matmul(
                    yT_ps,
                    wp_sb[:, h, :],
                    ov[(h, half)][0:64, :],
                    start=(h == 0),
                    stop=(h == H - 1),
                )
            nc.vector.tensor_scalar_add(yT_sb[:, nq], yT_ps, bp_sb)
            if half == 1:
                nc.sync.dma_start(out=yT_d[bi], in_=yT_sb)

        batches = [(rep, bi) for rep in range(reps) for bi in range(bb)]
        sts = {}
        sts[0] = alloc_batch(batches[0][1], batches[0][0] * 1000 + batches[0][1])
        # ---- load weights/biases ----
        wq_sb = wpool.tile([128, HID], dtqkv)
        wk_sb = wpool.tile([128, HID], dtqkv)
        wv_sb = wpool.tile([128, HID], dtqkv)
        nc.sync.dma_start(out=wq_sb, in_=wq_d[:, :])
        nc.sync.dma_start(out=wk_sb, in_=wk_d[:, :])
        nc.sync.dma_start(out=wv_sb, in_=wv_d[:, :])
        bqk_sb = wpool.tile([128, 8], f32)
        nc.sync.dma_start(out=bqk_sb, in_=bqk_d[:, :])
        bv_sb = wpool.tile([128, HID], f32)
        nc.sync.dma_start(
            out=bv_sb,
            in_=bv_d[:].unsqueeze(0).partition_broadcast(128),
        )
        wp_sb = wpool.tile([64, H, E], dtproj)
        nc.sync.dma_start(out=wp_sb, in_=wp_d[:, :, :])
        bp_sb = wpool.tile([E, 1], f32)
        nc.sync.dma_start(out=bp_sb, in_=bp_d[:, :])
        ones_sb = wpool.tile([128, H], f32)
        nc.vector.memset(ones_sb, 1.0)


        for g in range(4):
            emit_qkv_group(sts[0], g)
        pending = []  # (st, half) norm+proj deferred ~2 attention units

        def flush_pending(tail=False):
            if pending:
                st_p, half_p = pending.pop(0)
                if "yT" not in st_p:
                    st_p["yT"] = miscpool.tile([E, S], f32, tag="yT", name=f"yTsb_{st_p['b']}", bufs=1)
                emit_norm_proj(st_p, half_p, st_p["yT"], tail=tail)

        for i in range(len(batches)):
            st = sts.pop(i)
            for half in (0, 1):
                for hp in range(4):
                    emit_attention_unit(st, hp, half)
                    if half == 1 and i + 1 < len(batches):
                        if hp == 0:
                            rep, bi = batches[i + 1]
                            sts[i + 1] = alloc_batch(bi, rep * 1000 + bi)
                        emit_qkv_group(sts[i + 1], hp)
                    if hp == 1:
                        flush_pending()
                pending.append((st, half))
        while pending:
            flush_pending(tail=True)

    nc.compile()
    return nc


def _round_f32r(a):
    """Round fp32 to fp32r (11-bit mantissa, RNE) so DMA'd operands are
    pre-rounded as the BIR verifier requires for fp32r matmul consumers."""
    u = np.ascontiguousarray(a, np.float32).view(np.uint32)
    r = (u.astype(np.uint64) + 0x7FF + ((u >> 12) & 1)).astype(np.uint32) & np.uint32(
        0xFFFFF000
    )
    return r.view(np.float32)


def prep_inputs(x, w_qkv, b_qkv, w_proj, b_proj, bb=B // N_CORES, n_cores=N_CORES,
                variant=("f32r", "f32r", "f32r", "f32r")):
    """Host-side prep: permute/duplicate weights, transpose x, shard over cores."""
    x = np.asarray(x, np.float32)
    w_qkv = np.asarray(w_qkv, np.float32)
    b_qkv = np.asarray(b_qkv, np.float32)
    w_proj = np.asarray(w_proj, np.float32)
    b_proj = np.asarray(b_proj, np.float32)

    W = w_qkv.reshape(E, H, D, 3)
    wq = np.ascontiguousarray(W[..., 0].reshape(E, HID))
    wk = np.ascontiguousarray(W[..., 1].reshape(E, HID))
    wv = np.ascontiguousarray(W[..., 2].reshape(E, HID)) / 8.0
    wq_dup = np.concatenate([wq, wq], 0)  # [128, 512]
    wk_dup = np.concatenate([wk, wk], 0)
    wv_dup = np.concatenate([wv, wv], 0)

    Bq = b_qkv.reshape(H, D, 3)
    bq = Bq[..., 0].reshape(HID)
    bk = Bq[..., 1].reshape(HID)
    bv = Bq[..., 2].reshape(HID) / 8.0
    # bqk[p, qki*4 + t] = bias for qT/kT tile t partition p
    bqk = np.zeros((128, 8), np.float32)
    for t in range(4):
        bqk[:, 0 + t] = bq[128 * t : 128 * (t + 1)]
        bqk[:, 4 + t] = bk[128 * t : 128 * (t + 1)]

    wp = np.ascontiguousarray(w_proj.reshape(H, 64, E).transpose(1, 0, 2))  # [64, H, E]
    bp = np.ascontiguousarray(b_proj.reshape(E, 1))

    xT = x.transpose(0, 2, 1)  # [B, 64, S]
    xT_dup = np.ascontiguousarray(np.concatenate([xT, xT], axis=1))  # [B, 128, S]

    wq_dup = np.ascontiguousarray(wq_dup)
    wk_dup = np.ascontiguousarray(wk_dup)
    wv_dup = np.ascontiguousarray(wv_dup)
    if variant[2] == "f32r":  # qkv matmul operands
        xT_dup = _round_f32r(xT_dup)
        wq_dup, wk_dup, wv_dup = map(_round_f32r, (wq_dup, wk_dup, wv_dup))
    if variant[3] == "f32r":  # proj stationary
        wp = _round_f32r(wp)

    shared = {
        "wq": wq_dup, "wk": wk_dup, "wv": wv_dup,
        "bqk": bqk, "bv": np.ascontiguousarray(bv), "wp": wp, "bp": bp,
    }
    in_maps = []
    for c in range(n_cores):
        m = dict(shared)
        m["xT"] = np.ascontiguousarray(xT_dup[c * bb : (c + 1) * bb])
        in_maps.append(m)
    return in_maps


_CACHE = {}


def run(inputs, trace=False, variant=("f32r", "f32r", "f32r", "f32r")):
    from concourse.bass_utils import run_bass_kernel_spmd

    key = variant
    if key not in _CACHE:
        dt_e, dt_pv, dt_qkv, dt_proj = variant
        _CACHE[key] = build_nc(dt_e=dt_e, dt_pv=dt_pv, dt_qkv=dt_qkv, dt_proj=dt_proj)
    nc = _CACHE[key]
    in_maps = prep_inputs(**inputs, variant=variant)
    res = run_bass_kernel_spmd(nc, in_maps, core_ids=list(range(N_CORES)), trace=trace)
    bb = B // N_CORES
    y = np.concatenate(
        [res.results[c]["yT"].transpose(0, 2, 1) for c in range(N_CORES)], axis=0
    )
    return np.ascontiguousarray(y), res


def kernel(x, w_qkv, b_qkv, w_proj, b_proj):
    y, _ = run(dict(x=x, w_qkv=w_qkv, b_qkv=b_qkv, w_proj=w_proj, b_proj=b_proj))
    return y


# revision 47
# speedup vs baseline: 3.1759x; 3.1759x over previous
"""Trainium2 Bass kernel for nn_MultiHeadAttention (B=32, S=1024, E=64, H=8, D=64).

Strategy (per core; batch-parallel over 8 cores, 4 batches each):
  - Host-side numpy prep: permute w_qkv columns into per-head Q/K/V blocks,
    transpose x to xT (head-dim on partitions), duplicate operands across
    both 64-partition halves so pairs of K=64 matmuls run concurrently via
    PE row tiling. V is pre-scaled by 1/8 (the post-softmax scale).
  - On chip, everything stays in "transposed" layouts so no PE transposes
    are needed anywhere:
      qT/kT:   [2 heads * 64 d, 1024 nq]  (4 tiles per batch)
      V:       [128 nk-chunk, 8 heads * (64 v | 1)]  ones col => rowsums
      E^T:     [128 nk, 2 heads * 512 nq] per chunk -> exp (ScalarE) -> P^T
      P^T @ [V|1]: accumulates [65, 512] per head: rows 0..63 = (P V)/8,
                   row 64 = rowsum.  softmax normalization = multiply by
                   broadcast reciprocal of row 64 (no max subtraction:
                   energies are |E| < ~60, exp stays in fp32 range; softmax
                   is shift invariant so this matches the reference).
      proj:    per-head K=64 matmuls accumulate yT [64 e, nq]; output is
               returned transposed and fixed up on host.
  - Reciprocals are batched: rowsum rows are gathered via SBUF->SBUF DMA
    into a [16, 512] tile (one DVE reciprocal per batch), results DMA
    broadcast back across 64 partitions.
"""

import os
import sys

import numpy as np

_TRN_REPO = "/opt/trn_rl_repo"
if _TRN_REPO not in sys.path:
    sys.path.insert(0, _TRN_REPO)

B, S, E, H, D = 32, 1024, 64, 8, 64
HID = H * D  # 512
N_CORES = 8
NQH = 512  # nq half processed per psum tile


def build_nc(bb=B // N_CORES, dt_e="f32r", dt_pv="f32r", dt_qkv="f32r", dt_proj="f32r",
             reps=1):
    """Build the per-core Bass kernel. bb = batches per core."""
    import concourse.bass as bass
    import concourse.mybir as mybir
    import concourse.tile as tile
    from concourse import bacc
    from contextlib import ExitStack

    f32 = mybir.dt.float32
    f32r = mybir.dt.float32r
    bf16 = mybir.dt.bfloat16
    Exp = mybir.ActivationFunctionType.Exp

    def dt_of(key):
        return f32r if key == "f32r" else f32

    dte, dtpv, dtqkv, dtproj = dt_of(dt_e), dt_of(dt_pv), dt_of(dt_qkv), dt_of(dt_proj)

    nc = bacc.Bacc(None, target_bir_lowering=False)

    # ---- DRAM I/O (host-prepped layouts) ----
    xT_d = nc.dram_tensor("xT", [bb, 128, S], dtqkv, kind="ExternalInput")
    wq_d = nc.dram_tensor("wq", [128, HID], dtqkv, kind="ExternalInput")
    wk_d = nc.dram_tensor("wk", [128, HID], dtqkv, kind="ExternalInput")
    wv_d = nc.dram_tensor("wv", [128, HID], dtqkv, kind="ExternalInput")  # pre /8
    bqk_d = nc.dram_tensor("bqk", [128, 8], f32, kind="ExternalInput")
    bv_d = nc.dram_tensor("bv", [HID], f32, kind="ExternalInput")  # pre /8
    wp_d = nc.dram_tensor("wp", [64, H, E], dtproj, kind="ExternalInput")
    bp_d = nc.dram_tensor("bp", [E, 1], f32, kind="ExternalInput")
    yT_d = nc.dram_tensor("yT", [bb, E, S], f32, kind="ExternalOutput")

    with tile.TileContext(nc) as tc, ExitStack() as ctx:
        wpool = ctx.enter_context(tc.tile_pool(name="weights", bufs=1))
        qkpool = ctx.enter_context(tc.tile_pool(name="qk", bufs=2))
        vpool = ctx.enter_context(tc.tile_pool(name="v", bufs=2))
        ptpool = ctx.enter_context(tc.tile_pool(name="pt", bufs=4))
        ovpool = ctx.enter_context(tc.tile_pool(name="ov", bufs=16))
        rbpool = ctx.enter_context(tc.tile_pool(name="rb", bufs=4))
        miscpool = ctx.enter_context(tc.tile_pool(name="misc", bufs=2))
        psum_e = ctx.enter_context(tc.tile_pool(name="psum_e", bufs=2, space="PSUM"))
        psum_s = ctx.enter_context(tc.tile_pool(name="psum_s", bufs=4, space="PSUM"))
        drampool = ctx.enter_context(tc.tile_pool(name="dram", bufs=2, space="DRAM"))

        def alloc_batch(bi, b):
            xT_sb = qkpool.tile([128, S], dtqkv, tag="xT", name=f"xT_{b}")
            nc.sync.dma_start(out=xT_sb[:, 0:NQH], in_=xT_d[bi][:, 0:NQH])
            nc.sync.dma_start(out=xT_sb[:, NQH:S], in_=xT_d[bi][:, NQH:S])
            qT = [qkpool.tile([128, S], dte, tag=f"qT{t}", name=f"qT{t}_{b}") for t in range(4)]
            kT = [qkpool.tile([128, S], dte, tag=f"kT{t}", name=f"kT{t}_{b}") for t in range(4)]
            v_nat = [vpool.tile([128, H * 65], dtpv, tag=f"v{c}", name=f"v{c}_{b}") for c in range(8)]
            rsd = [
                drampool.tile([H, NQH], dtproj, tag=f"rs_dram{hf}", name=f"rsd_{b}_{hf}")
                for hf in range(2)
            ]
            return dict(bi=bi, b=b, xT=xT_sb, qT=qT, kT=kT, v=v_nat, ov={}, rsd=rsd)

        def emit_qk_pair(st, qki, tp):
            w_sb = (wq_sb, wk_sb)[qki]
            dst = (st["qT"], st["kT"])[qki]
            xT_sb, b = st["xT"], st["b"]
            for half in range(2):
                nq = slice(half * NQH, (half + 1) * NQH)
                ps_e = psum_s.tile([128, NQH], f32, tag="small", name=f"psqkv_e{b}_{qki}{tp}{half}")
                ps_o = psum_s.tile([128, NQH], f32, tag="small", name=f"psqkv_o{b}_{qki}{tp}{half}")
                nc.tensor.matmul(ps_e, w_sb[0:64, 128 * tp : 128 * (tp + 1)], xT_sb[0:64, nq])
                nc.tensor.matmul(ps_o, w_sb[64:128, 128 * (tp + 1) : 128 * (tp + 2)], xT_sb[64:128, nq])
                nc.vector.tensor_scalar_add(
                    dst[tp][:, nq], ps_e, bqk_sb[:, qki * 4 + tp : qki * 4 + tp + 1]
                )
                nc.vector.tensor_scalar_add(
                    dst[tp + 1][:, nq], ps_o, bqk_sb[:, qki * 4 + tp + 1 : qki * 4 + tp + 2]
                )

        def emit_v_pair(st, cp):
            xT_sb, v_nat, b = st["xT"], st["v"], st["b"]
            ps_e = psum_s.tile([128, HID], f32, tag="small", name=f"psv_e{b}_{cp}")
            ps_o = psum_s.tile([128, HID], f32, tag="small", name=f"psv_o{b}_{cp}")
            nc.tensor.matmul(ps_e, xT_sb[0:64, 128 * cp : 128 * (cp + 1)], wv_sb[0:64, :])
            nc.tensor.matmul(ps_o, xT_sb[64:128, 128 * (cp + 1) : 128 * (cp + 2)], wv_sb[64:128, :])
            for c, pss in ((cp, ps_e), (cp + 1, ps_o)):
                vdst = v_nat[c].rearrange("p (h c65) -> p h c65", c65=65)
                nc.vector.tensor_tensor(
                    vdst[:, :, 0:64],
                    pss.rearrange("p (h d) -> p h d", d=64),
                    bv_sb.rearrange("p (h d) -> p h d", d=64),
                    mybir.AluOpType.add,
                )
                nc.vector.tensor_copy(vdst[:, :, 64], ones_sb)

        def emit_qkv_group(st, g):
            # startup-friendly order: heads 0-3 weights, all V, heads 4-7
            if g == 0:
                emit_qk_pair(st, 0, 0)
                emit_qk_pair(st, 1, 0)
            elif g == 1:
                emit_v_pair(st, 0)
                emit_v_pair(st, 2)
            elif g == 2:
                emit_v_pair(st, 4)
                emit_v_pair(st, 6)
            else:
                emit_qk_pair(st, 0, 2)
                emit_qk_pair(st, 1, 2)

        def emit_attention_unit(st, hp, half):
            qT, kT, v_nat, b = st["qT"], st["kT"], st["v"], st["b"]
            nq = slice(half * NQH, (half + 1) * NQH)
            oT_e = psum_s.tile([65, NQH], f32, tag="small", name=f"oTe_{b}_{hp}_{half}")
            oT_o = psum_s.tile([65, NQH], f32, tag="small", name=f"oTo_{b}_{hp}_{half}")
            for c in range(8):
                eT = psum_e.tile([128, 2 * NQH], f32, tag="eT", name=f"eT_{b}_{hp}_{half}_{c}")
                nc.tensor.matmul(
                    eT[:, 0:NQH], kT[hp][0:64, 128 * c : 128 * (c + 1)], qT[hp][0:64, nq]
                )
                nc.tensor.matmul(
                    eT[:, NQH : 2 * NQH],
                    kT[hp][64:128, 128 * c : 128 * (c + 1)],
                    qT[hp][64:128, nq],
                )
                pt = ptpool.tile([128, 2 * NQH], dtpv, tag="pt", name=f"pt_{b}_{hp}_{half}_{c}")
                nc.scalar.activation(pt, eT, Exp)
                nc.tensor.matmul(
                    oT_e,
                    v_nat[c][:, (2 * hp) * 65 : (2 * hp) * 65 + 65],
                    pt[:, 0:NQH],
                    start=(c == 0),
                    stop=(c == 7),
                )
                nc.tensor.matmul(
                    oT_o,
                    v_nat[c][:, (2 * hp + 1) * 65 : (2 * hp + 1) * 65 + 65],
                    pt[:, NQH : 2 * NQH],
                    start=(c == 0),
                    stop=(c == 7),
                )
            rs_dram = st["rsd"][half]
            for par, oT in ((0, oT_e), (1, oT_o)):
                h = 2 * hp + par
                t = ovpool.tile([65, NQH], dtproj, tag="ov", name=f"ov_{b}_{h}_{half}")
                nc.vector.tensor_copy(t, oT)
                st["ov"][(h, half)] = t
                # gather rowsum rows via DRAM as units complete (SBUF APs may
                # only start at partition 0/32/64/96, so collection detours
                # through DRAM)
                nc.sync.dma_start(out=rs_dram[h : h + 1, :], in_=t[64:65, :])

        def emit_norm_proj(st, half, yT_sb, tail=False):
            ov, b, bi = st["ov"], st["b"], st["bi"]
            rs_dram = st["rsd"][half]
            dma_eng = nc.scalar if tail else nc.sync
            rs = miscpool.tile([H, NQH], dtproj, tag="rs", name=f"rs_{b}_{half}")
            dma_eng.dma_start(out=rs, in_=rs_dram[:, :])
            rcp = miscpool.tile([H, NQH], f32, tag="rcp", name=f"rcp_{b}_{half}")
            rscr = miscpool.tile([H, NQH], f32, tag="rscr", name=f"rscr_{b}_{half}", bufs=1)
            nc.vector.reciprocal_approx_accurate(rcp, rs.bitcast(f32), rscr)
            rcp_dram = drampool.tile([H, NQH], f32, tag=f"rcp_dram{half}", name=f"rcpd_{b}_{half}")
            dma_eng.dma_start(out=rcp_dram, in_=rcp)
            nq = slice(half * NQH, (half + 1) * NQH)
            for h in range(H):
                recB = rbpool.tile([64, NQH], f32, tag="recB", name=f"recB_{b}_{h}_{half}")
                dma_eng.dma_start(
                    out=recB, in_=rcp_dram[h : h + 1, :].partition_broadcast(64)
                )
                nc.vector.tensor_tensor(
                    ov[(h, half)][0:64, :],
                    ov[(h, half)][0:64, :],
                    recB,
                    mybir.AluOpType.mult,
                )
            yT_ps = psum_s.tile([E, NQH], f32, tag="small", name=f"yTps_{b}_{half}")
            for h in range(H):
                nc.tensor.matmul(
                    yT_ps,
                    wp_sb[:, h, :],
                    ov[(h, half)][0:64, :],
                    start=(h == 0),
                    stop=(h == H - 1),
                )
            nc.vector.tensor_scalar_add(yT_sb[:, nq], yT_ps, bp_sb)
            if half == 1:
                nc.sync.dma_start(out=yT_d[bi], in_=yT_sb)

        batches = [(rep, bi) for rep in range(reps) for bi in range(bb)]
        sts = {}
        sts[0] = alloc_batch(batches[0][1], batches[0][0] * 1000 + batches[0][1])
        # ---- load weights/biases ----
        wq_sb = wpool.tile([128, HID], dtqkv)
        wk_sb = wpool.tile([128, HID], dtqkv)
        wv_sb = wpool.tile([128, HID], dtqkv)
        nc.sync.dma_start(out=wq_sb, in_=wq_d[:, :])
        nc.sync.dma_start(out=wk_sb, in_=wk_d[:, :])
        nc.sync.dma_start(out=wv_sb, in_=wv_d[:, :])
        bqk_sb = wpool.tile([128, 8], f32)
        nc.sync.dma_start(out=bqk_sb, in_=bqk_d[:, :])
        bv_sb = wpool.tile([128, HID], f32)
        nc.sync.dma_start(
            out=bv_sb,
            in_=bv_d[:].unsqueeze(0).partition_broadcast(128),
        )
        wp_sb = wpool.tile([64, H, E], dtproj)
        nc.sync.dma_start(out=wp_sb, in_=wp_d[:, :, :])
        bp_sb = wpool.tile([E, 1], f32)
        nc.sync.dma_start(out=bp_sb, in_=bp_d[:, :])
        ones_sb = wpool.tile([128, H], f32)
        nc.vector.memset(ones_sb, 1.0)


        for g in range(4):
            emit_qkv_group(sts[0], g)
        pending = []  # (st, half) norm+proj deferred ~2 attention units

        def flush_pending(tail=False):
            if pending:
                st_p, half_p = pending.pop(0)
                if "yT" not in st_p:
                    st_p["yT"] = miscpool.tile([E, S], f32, tag="yT", name=f"yTsb_{st_p['b']}", bufs=1)
                emit_norm_proj(st_p, half_p, st_p["yT"], tail=tail)

        for i in range(len(batches)):
            st = sts.pop(i)
            for half in (0, 1):
                for hp in range(4):
                    emit_attention_unit(st, hp, half)
                    if half == 1 and i + 1 < len(batches):
                        if hp == 0:
                            rep, bi = batches[i + 1]
                            sts[i + 1] = alloc_batch(bi, rep * 1000 + bi)
                        emit_qkv_group(sts[i + 1], hp)
                    if hp == 1:
                        flush_pending()
                pending.append((st, half))
        while pending:
            flush_pending(tail=True)

    nc.compile()
    return nc


def _round_f32r(a):
    """Round fp32 to fp32r (11-bit mantissa, RNE) so DMA'd operands are
    pre-rounded as the BIR verifier requires for fp32r matmul consumers."""
    u = np.ascontiguousarray(a, np.float32).view(np.uint32)
    r = (u.astype(np.uint64) + 0x7FF + ((u >> 12) & 1)).astype(np.uint32) & np.uint32(
        0xFFFFF000
    )
    return r.view(np.float32)


def prep_inputs(x, w_qkv, b_qkv, w_proj, b_proj, bb=B // N_CORES, n_cores=N_CORES,
                variant=("f32r", "f32r", "f32r", "f32r")):
    """Host-side prep: permute/duplicate weights, transpose x, shard over cores."""
    x = np.asarray(x, np.float32)
    w_qkv = np.asarray(w_qkv, np.float32)
    b_qkv = np.asarray(b_qkv, np.float32)
    w_proj = np.asarray(w_proj, np.float32)
    b_proj = np.asarray(b_proj, np.float32)

    W = w_qkv.reshape(E, H, D, 3)
    wq = np.ascontiguousarray(W[..., 0].reshape(E, HID))
    wk = np.ascontiguousarray(W[..., 1].reshape(E, HID))
    wv = np.ascontiguousarray(W[..., 2].reshape(E, HID)) / 8.0
    wq_dup = np.concatenate([wq, wq], 0)  # [128, 512]
    wk_dup = np.concatenate([wk, wk], 0)
    wv_dup = np.concatenate([wv, wv], 0)

    Bq = b_qkv.reshape(H, D, 3)
    bq = Bq[..., 0].reshape(HID)
    bk = Bq[..., 1].reshape(HID)
    bv = Bq[..., 2].reshape(HID) / 8.0
    # bqk[p, qki*4 + t] = bias for qT/kT tile t partition p
    bqk = np.zeros((128, 8), np.float32)
    for t in range(4):
        bqk[:, 0 + t] = bq[128 * t : 128 * (t + 1)]
        bqk[:, 4 + t] = bk[128 * t : 128 * (t + 1)]

    wp = np.ascontiguousarray(w_proj.reshape(H, 64, E).transpose(1, 0, 2))  # [64, H, E]
    bp = np.ascontiguousarray(b_proj.reshape(E, 1))

    xT = x.transpose(0, 2, 1)  # [B, 64, S]
    xT_dup = np.ascontiguousarray(np.concatenate([xT, xT], axis=1))  # [B, 128, S]

    wq_dup = np.ascontiguousarray(wq_dup)
    wk_dup = np.ascontiguousarray(wk_dup)
    wv_dup = np.ascontiguousarray(wv_dup)
    if variant[2] == "f32r":  # qkv matmul operands
        xT_dup = _round_f32r(xT_dup)
        wq_dup, wk_dup, wv_dup = map(_round_f32r, (wq_dup, wk_dup, wv_dup))
    if variant[3] == "f32r":  # proj stationary
        wp = _round_f32r(wp)

    shared = {
        "wq": wq_dup, "wk": wk_dup, "wv": wv_dup,
        "bqk": bqk, "bv": np.ascontiguousarray(bv), "wp": wp, "bp": bp,
    }
    in_maps = []
    for c in range(n_cores):
        m = dict(shared)
        m["xT"] = np.ascontiguousarray(xT_dup[c * bb : (c + 1) * bb])
        in_maps.append(m)
    return in_maps


_CACHE = {}


def run(inputs, trace=False, variant=("f32r", "f32r", "f32r", "f32r")):
    from concourse.bass_utils import run_bass_kernel_spmd

    key = variant
    if key not in _CACHE:
        dt_e, dt_pv, dt_qkv, dt_proj = variant
        _CACHE[key] = build_nc(dt_e=dt_e, dt_pv=dt_pv, dt_qkv=dt_qkv, dt_proj=dt_proj)
    nc = _CACHE[key]
    in_maps = prep_inputs(**inputs, variant=variant)
    res = run_bass_kernel_spmd(nc, in_maps, core_ids=list(range(N_CORES)), trace=trace)
    bb = B // N_CORES
    y = np.concatenate(
        [res.results[c]["yT"].transpose(0, 2, 1) for c in range(N_CORES)], axis=0
    )
    return np.ascontiguousarray(y), res


def kernel(x, w_qkv, b_qkv, w_proj, b_proj):
    y, _ = run(dict(x=x, w_qkv=w_qkv, b_qkv=b_qkv, w_proj=w_proj, b_proj=b_proj))
    return y
